# revision 1
# baseline (speedup 1.0000x reference)
"""Trainium2 Bass kernel for nn_MultiHeadAttention_23467701305746.

Reference computation (batch 8, seq 1024, hidden 512, 16 heads x 32):
  q/k/v = relu(x @ W + b); scores = q k^T / sqrt(32); attn = softmax(scores)
  out = attn @ v + x;  BatchNorm1d over (batch, seq) per channel, eps=1e-3.

Sharding: data-parallel over batch, 1 batch element per NeuronCore (8 cores).
BatchNorm batch statistics are combined with a tiny (4 KB) AllReduce.

Per-core dataflow (layout "B" = channels on partitions for attention math):
  x [1024s, 512c] --PE transpose--> xT [512c, 1024s] (bf16)
  qT/kT = relu(W^T x + b) bf16 layout B; v = relu(x W + b) bf16 layout A,
  stored as [k, head, 33] with a built-in ones column.
  Per head pair (row-strip packed, 2 heads per PE pass):
    S^T[k,q] = kT_h^T qT_h; expS = exp(S/sqrt(d)) on ACT (bf16 out)
    One 33-col matmul per head gives U^T rows AND the softmax denominator
    row in a single pass (stationary = [v_h | 1]).
  U^T chunks --PE bf16 transpose--> layout A; batched reciprocal + broadcast
  multiply normalizes; residual add + BN partial sums (ones-matmul)
  -> AllReduce -> scale/shift (Quake rsqrt on DVE; no ACT table swap) -> out.

Engine placement: big strided DMAs (one per weight matrix) land in f32
staging; one ACT copy converts each to bf16 (fp32r cannot be produced by
bitcast - the verifier requires rounded producers). ACT also does all
relu+bias epilogues and PSUM evacuations; DVE keeps 2-input elementwise ops,
batched over [128, 8, 512] tiles to amortize instruction overhead.
Projection chunks are interleaved with attention pairs in emission order so
the in-order PE/ACT queues overlap prep and attention work.
"""

import math
import numpy as np
from contextlib import ExitStack

import concourse.bass as bass
import concourse.tile as tile
from concourse import bacc, mybir
from concourse.bass_utils import run_bass_kernel_spmd
from concourse.masks import make_identity

F32 = mybir.dt.float32
F32R = mybir.dt.float32r
BF16 = mybir.dt.bfloat16
I32 = mybir.dt.int32
OP = mybir.AluOpType
AF = mybir.ActivationFunctionType

N_CORES = 8
S = 1024          # sequence length per core (= per batch element)
H = 512           # hidden
NH = 16           # heads
D = 32            # head dim
KC = H // 128     # 4 contraction chunks over hidden
SC = S // 128     # 8 chunks over sequence
QH = S // 512     # 2 query halves (N=512 fp32 matmul limit)
BN_EPS = 1e-3
INV_SQRT_D = 1.0 / math.sqrt(D)
N_ROWS_TOTAL = 8 * S  # BN stats denominator (batch*seq)
QUAKE_C = 0x5F3759DF


def emit_body(nc, tc, outer_ctx, tens, with_tail=True, parts=("prep", "attn")):
    x, wq, bq, wk, bk, wv, bv, gamma, beta, out = tens
    ctx = outer_ctx.enter_context(ExitStack())

    const = ctx.enter_context(tc.tile_pool(name="const", bufs=1))
    xpool = ctx.enter_context(tc.tile_pool(name="xpool", bufs=1))
    qkp = ctx.enter_context(tc.tile_pool(name="qkp", bufs=1))
    vpool = ctx.enter_context(tc.tile_pool(name="vpool", bufs=1))
    statp = ctx.enter_context(tc.tile_pool(name="statp", bufs=1))
    psum = ctx.enter_context(tc.tile_pool(name="psum", bufs=1, space="PSUM"))
    psum2 = ctx.enter_context(tc.tile_pool(name="psum2", bufs=2, space="PSUM"))

    # ---- constants ----
    ident = const.tile([128, 128], F32, tag="ident", name="ident")
    make_identity(nc, ident[:, :])
    ident_bf = const.tile([128, 128], BF16, tag="ident_bf", name="ident_bf")
    nc.vector.tensor_copy(ident_bf[:, :], ident[:, :])
    ones_f32 = const.tile([128, 512], F32, tag="ones_f32", name="ones_f32")
    nc.vector.memset(ones_f32[:, :], 1.0)
    exp_warm = const.tile([1, 1], F32, tag="exp_warm", name="exp_warm")
    nc.scalar.activation(exp_warm[:, :], ones_f32[0:1, 0:1], AF.Exp)
    warm_mv = const.tile([128, 512], BF16, tag="warm_mv", name="warm_mv")
    nc.vector.memset(warm_mv[:, :], 0.0)
    wps = psum2.tile([128, 512], F32, tag="tps", name="tps")
    for _ in range(10):
        nc.tensor.matmul(wps[:, :], ident_bf[:, :], warm_mv[:, :],
                         start=True, stop=True)
    ones_row_b = const.tile([1, 128], BF16, tag="ones_row_b", name="ones_row_b")  # K=1 lhsT
    nc.vector.tensor_copy(ones_row_b[:, :], ones_f32[0:1, 0:128])
    ones_row_r = const.tile([1, 128], F32R, tag="ones_row_r", name="ones_row_r")
    nc.vector.tensor_copy(ones_row_r[:, :], ones_f32[0:1, 0:128])
    ones_col_r = const.tile([128, 1], F32R, tag="ones_col_r", name="ones_col_r")
    nc.vector.tensor_copy(ones_col_r[:, :], ones_f32[:, 0:1])
    gamma_sb = const.tile([1, 512], F32, tag="gamma", name="gamma")
    beta_sb = const.tile([1, 512], F32, tag="beta", name="beta")

    # ---- x load (two strided DMAs so transposes start early) ----
    x_sb = xpool.tile([128, SC, 512], F32, tag="x_sb", name="x_sb")
    xr = x[:, :].rearrange("(r p) c -> p r c", p=128)
    for q4 in range(4):
        lo, hi = q4 * (SC // 4), (q4 + 1) * (SC // 4)
        nc.sync.dma_start(x_sb[:, lo:hi, :], xr[:, lo:hi, :])

    qT = [qkp.tile([128, S], BF16, tag=f"qT{c}", name=f"qT{c}") for c in range(KC)]
    kT = [qkp.tile([128, S], BF16, tag=f"kT{c}", name=f"kT{c}") for c in range(KC)]
    # v is stored as [128k, head, 33]: columns 0:32 are v_h, column 32 is 1.0
    # so one U matmul per head yields U rows plus the softmax denominator row
    v_r = [vpool.tile([128, NH, 33], BF16, tag=f"v{kv}", name=f"v{kv}") for kv in range(SC)]

    if "prep" not in parts:
        nc.sync.dma_start(gamma_sb[:, :], gamma[:].unsqueeze(0))
        nc.sync.dma_start(beta_sb[:, :], beta[:].unsqueeze(0))
        # bisection mode: fill qT/kT/v with memset instead of real projections
        for c in range(KC):
            nc.vector.memset(qT[c][:, :], 0.01)
            nc.vector.memset(kT[c][:, :], 0.01)
        for kv in range(SC):
            nc.vector.memset(v_r[kv][:, :, :], 0.5)
            nc.vector.tensor_copy(v_r[kv][:, :, 32:33], ones_f32[:, 0:NH].unsqueeze(2))
    # ---- pools (prep + attention live together for the interleave) ----
    expp = ctx.enter_context(tc.tile_pool(name="expp", bufs=2))
    usb = ctx.enter_context(tc.tile_pool(name="usb", bufs=2))
    outp = ctx.enter_context(tc.tile_pool(name="outp", bufs=1))
    tmpp = ctx.enter_context(tc.tile_pool(name="tmpp", bufs=2))
    outA = outp.tile([128, SC, 512], F32R, tag="outA", name="outA")

    have_prep = "prep" in parts
    have_attn = "attn" in parts

    if have_prep:
        wpool = ctx.enter_context(tc.tile_pool(name="wpool", bufs=1))
        wstg = ctx.enter_context(tc.tile_pool(name="wstg", bufs=1))
        xtp = ctx.enter_context(tc.tile_pool(name="xtp", bufs=1))

        # one strided DMA per weight matrix into f32 staging [128p, kc, 512c]
        # (p = contraction sub-index), then one big ACT copy to bf16
        w_st, w_r = {}, {}
        for name, wt in (("q", wq), ("k", wk), ("v", wv)):
            st = wstg.tile([128, KC, 512], F32, tag=f"ws{name}", name=f"ws{name}")
            nc.sync.dma_start(st[:, :, :],
                              wt[:, :].rearrange("(k p) c -> p k c", p=128))
            w_st[name] = st
            w_r[name] = wpool.tile([128, KC, 512], BF16, tag=f"w{name}",
                                   name=f"w{name}")
        bqk = wpool.tile([128, 2, KC], F32, tag="bqk", name="bqk")
        nc.sync.dma_start(bqk[:, 0, :], bq[:].rearrange("(k p) -> p k", p=128))
        nc.sync.dma_start(bqk[:, 1, :], bk[:].rearrange("(k p) -> p k", p=128))
        bv_sb = wpool.tile([1, 512], F32, tag="bv_sb", name="bv_sb")
        nc.sync.dma_start(bv_sb[:, :], bv[:].unsqueeze(0))
        nc.sync.dma_start(gamma_sb[:, :], gamma[:].unsqueeze(0))
        nc.sync.dma_start(beta_sb[:, :], beta[:].unsqueeze(0))
        bv_b = wpool.tile([1, 512], BF16, tag="bv_b", name="bv_b")
        nc.vector.tensor_copy(bv_b[:, :], bv_sb[:, :])

        def conv_w(name):
            nc.scalar.copy(w_r[name][:, :, :].rearrange("p a b -> p (a b)"),
                           w_st[name][:, :, :].rearrange("p a b -> p (a b)"))

        # q/k weight converts first on ACT (they gate the first projections)
        conv_w("q")
        conv_w("k")

        # x transpose -> xT (bf16); batched PSUM evacuation on ACT
        xT = [xtp.tile([128, S], BF16, tag=f"xT{c}", name=f"xT{c}") for c in range(KC)]
        for c in range(KC):
            for half in range(2):
                tp4 = psum2.tile([128, 4, 128], F32, tag="tps", name="tps")
                for rr in range(4):
                    r = half * 4 + rr
                    nc.tensor.transpose(tp4[:, rr, :],
                                        x_sb[:, r, c * 128:(c + 1) * 128],
                                        ident[:, :])
                nc.scalar.copy(
                    xT[c][:, half * 512:(half + 1) * 512],
                    tp4[:, :, :].rearrange("p a b -> p (a b)"))

        def proj_qk(oc):
            # projections q,k chunk oc (layout B); relu+bias fused on ACT
            for wi, (wkey, dest) in enumerate((("q", qT), ("k", kT))):
                for sh in range(QH):
                    ps = psum2.tile([128, 512], F32, tag="tps", name="tps")
                    for kc in range(KC):
                        nc.tensor.matmul(
                            ps[:, :],
                            w_r[wkey][:, kc, oc * 128:(oc + 1) * 128],
                            xT[kc][:, sh * 512:(sh + 1) * 512],
                            start=(kc == 0), stop=(kc == KC - 1))
                    nc.scalar.activation(
                        dest[oc][:, sh * 512:(sh + 1) * 512],
                        ps[:, :], AF.Relu, bias=bqk[:, wi, oc:oc + 1],
                        scale=1.0)

        def proj_v():
            # v (layout A); bias via ones-matmul, relu on ACT
            conv_w("v")
            for kv in range(SC):
                ps = psum2.tile([128, 512], F32, tag="tps", name="tps")
                for kc in range(KC):
                    nc.tensor.matmul(ps[:, :], xT[kc][:, kv * 128:(kv + 1) * 128],
                                     w_r["v"][:, kc, :],
                                     start=(kc == 0), stop=False)
                nc.tensor.matmul(ps[:, :], ones_row_b[:, :], bv_b[:, :],
                                 start=False, stop=True)
                nc.scalar.activation(
                    v_r[kv][:, :, 0:32],
                    ps[:, :].rearrange("p (h d) -> p h d", h=NH), AF.Relu)
                nc.vector.tensor_copy(v_r[kv][:, :, 32:33], ones_f32[:, 0:NH].unsqueeze(2))

    def emit_pair_mm(hp):
        h0, h1 = 2 * hp, 2 * hp + 1
        ch = h0 // 4                 # qT/kT chunk holding these heads
        p0 = (h0 % 4) * 32           # partition base of h0 within chunk
        p1 = (h1 % 4) * 32
        ups = psum.tile([128, S], F32, tag="u_ps", name="u_ps")

        def emit_scores(kc, qh):
            # half-size chunk with a double-buffered PSUM tag so the PE can
            # run one chunk ahead of the ACT exp instead of stalling
            sps = psum.tile([128, 2, 512], F32, tag="score_ps",
                            name="score_ps", bufs=2)
            for j, pb in ((0, p0), (1, p1)):
                nc.tensor.matmul(
                    sps[:, j, :],
                    kT[ch][pb:pb + 32, kc * 128:(kc + 1) * 128],
                    qT[ch][pb:pb + 32, qh * 512:(qh + 1) * 512],
                    start=True, stop=True, tile_position=(pb, 0))
            ex = expp.tile([128, 2, 512], BF16, tag="expS", name="expS",
                           bufs=4)
            nc.scalar.activation(
                ex[:, :, :].rearrange("p a c -> p (a c)"),
                sps[:, :, :].rearrange("p a c -> p (a c)"),
                AF.Exp, scale=INV_SQRT_D)
            return ex

        def emit_u(kc, qh, ex):
            # one 33-col matmul per head: rows 0:32 = U_h, row 32 = rowsum
            st, sp = (kc == 0), (kc == SC - 1)
            q0, q1 = qh * 512, (qh + 1) * 512
            nc.tensor.matmul(ups[0:33, q0:q1], v_r[kc][:, h0, :],
                             ex[:, 0, :], start=st, stop=sp,
                             tile_position=(0, 0))
            nc.tensor.matmul(ups[64:97, q0:q1], v_r[kc][:, h1, :],
                             ex[:, 1, :], start=st, stop=sp,
                             tile_position=(0, 64))

        chunks = [(kc, qh) for kc in range(SC) for qh in range(QH)]
        prev_c, prev_ex = chunks[0], emit_scores(*chunks[0])
        for cur in chunks[1:]:
            cur_ex = emit_scores(*cur)
            emit_u(prev_c[0], prev_c[1], prev_ex)
            prev_c, prev_ex = cur, cur_ex
        emit_u(prev_c[0], prev_c[1], prev_ex)

        # evacuate U (rows [0:32]=U_h0, [32]=rowsum_h0, [64:96]=U_h1,
        # [96]=rowsum_h1) to bf16 SBUF on ACT; transposes happen in _fin
        us = usb.tile([128, S], BF16, tag="u_sb", name="u_sb")
        nc.scalar.copy(us[:, :], ups[:, :])
        return us

    def emit_pair_fin(hp, us):
        # transpose each seq chunk to layout A and divide by rowsum column;
        # all 8 bf16 transposes of the pair pack into one PSUM bank tile
        tpb = psum2.tile([128, SC, 128], BF16, tag="tps", name="tps")
        for sc in range(SC):
            nc.tensor.transpose(tpb[:, sc, :], us[:, sc * 128:(sc + 1) * 128],
                                ident_bf[:, :])
        rsr = statp.tile([128, SC, 2], F32, tag="rsr", name="rsr", bufs=2)
        for j, (vcol, rcol) in enumerate(((0, 32), (64, 96))):
            nc.vector.reciprocal(rsr[:, :, j:j + 1], tpb[:, :, rcol:rcol + 1])
            col = 64 * hp + 32 * j
            in0, in1 = bass.broadcast_tensor_aps(
                tpb[:, :, vcol:vcol + 32], rsr[:, :, j:j + 1])
            nc.vector.tensor_mul(outA[:, :, col:col + 32], in0, in1)

    # interleaved emission: projections feed attention pairs chunk by chunk
    # so PE projection work hides under ACT exp work (per-engine queues are
    # in-order; emission order controls overlap). Each pair's evac
    # transposes are deferred past the next projection chunk so the PE
    # doesn't idle waiting on the ACT us-copy.
    if have_prep and have_attn:
        proj_qk(0)
        proj_v()
        usd = {}
        usd[0] = emit_pair_mm(0)
        proj_qk(1)
        usd[1] = emit_pair_mm(1)
        emit_pair_fin(0, usd[0])
        proj_qk(2)
        usd[2] = emit_pair_mm(2)
        emit_pair_fin(1, usd[1])
        proj_qk(3)
        usd[3] = emit_pair_mm(3)
        emit_pair_fin(2, usd[2])
        for hp in range(4, NH // 2):
            usd[hp] = emit_pair_mm(hp)
            emit_pair_fin(hp - 1, usd[hp - 1])
        emit_pair_fin(NH // 2 - 1, usd[NH // 2 - 1])
    elif have_prep:
        for oc in range(KC):
            proj_qk(oc)
        proj_v()
    elif have_attn:
        prev = emit_pair_mm(0)
        for hp in range(1, NH // 2):
            nxt = emit_pair_mm(hp)
            emit_pair_fin(hp - 1, prev)
            prev = nxt
        emit_pair_fin(NH // 2 - 1, prev)

    if not have_attn:
        for sc in range(SC):
            nc.vector.tensor_scalar(out=outA[:, sc, :], in0=ones_f32[:, :],
                                    scalar1=0.1, scalar2=None, op0=OP.mult)
    # ---- tail: residual, BN stats + AllReduce, scale/shift, output ----
    sum_ps = psum.tile([1, 512], F32, tag="score_ps", name="sum_ps", bufs=2)
    sq_ps = psum.tile([1, 512], F32, tag="u_ps", name="sq_ps")
    nc.vector.tensor_add(outA[:, :, :], outA[:, :, :], x_sb[:, :, :])
    sq = tmpp.tile([128, SC, 512], F32R, tag="sq", name="sq", bufs=1)
    nc.vector.tensor_mul(sq[:, :, :], outA[:, :, :], outA[:, :, :])
    for sc in range(SC):
        nc.tensor.matmul(sum_ps[:, :], ones_col_r, outA[:, sc, :],
                         start=(sc == 0), stop=(sc == SC - 1))
        nc.tensor.matmul(sq_ps[:, :], ones_col_r, sq[:, sc, :],
                         start=(sc == 0), stop=(sc == SC - 1))

    if not with_tail:
        # timing-only build: skip collective (banned in control flow); apply
        # a dummy copy so outA is still consumed.
        t2 = tmpp.tile([128, SC, 512], F32, tag="t2", name="t2", bufs=1)
        nc.vector.tensor_copy(t2[:, :, :], outA[:, :, :])
        nc.sync.dma_start(out[:, :].rearrange("(r p) c -> p r c", p=128),
                          t2[:, :, :])
        ctx.close()
        return

    dram = ctx.enter_context(tc.tile_pool(name="dram", bufs=1, space="DRAM"))
    stats_sb = statp.tile([1, 1024], F32, tag="stats_sb", name="stats_sb")
    nc.vector.tensor_copy(stats_sb[:, 0:512], sum_ps[:, :])
    nc.vector.tensor_copy(stats_sb[:, 512:1024], sq_ps[:, :])
    cc_in = dram.tile([1, 1024], F32)
    cc_out = dram.tile([1, 1024], F32)
    nc.sync.dma_start(cc_in[:, :], stats_sb[:, :])
    nc.gpsimd.collective_compute(
        "AllReduce", OP.add,
        replica_groups=[list(range(N_CORES))],
        ins=[cc_in[:, :].opt()], outs=[cc_out[:, :].opt()])
    gstats = statp.tile([1, 1024], F32, tag="gstats", name="gstats")
    nc.sync.dma_start(gstats[:, :], cc_out[:, :])
    mean = statp.tile([1, 512], F32, tag="mean", name="mean")
    nc.vector.tensor_scalar(out=mean[:, :], in0=gstats[:, 0:512],
                            scalar1=1.0 / N_ROWS_TOTAL, scalar2=None,
                            op0=OP.mult)
    esq = statp.tile([1, 512], F32, tag="esq", name="esq")
    nc.vector.tensor_scalar(out=esq[:, :], in0=gstats[:, 512:1024],
                            scalar1=1.0 / N_ROWS_TOTAL, scalar2=None,
                            op0=OP.mult)
    var = statp.tile([1, 512], F32, tag="var", name="var")
    nc.vector.tensor_mul(var[:, :], mean[:, :], mean[:, :])
    nc.vector.tensor_sub(var[:, :], esq[:, :], var[:, :])
    # ve = var + eps; y = rsqrt(ve) via Quake bit-trick + 2 Newton steps
    # (keeps the whole tail off ACT so the exp table set never swaps)
    ve = statp.tile([1, 512], F32, tag="ve", name="ve")
    nc.vector.tensor_scalar(out=ve[:, :], in0=var[:, :], scalar1=BN_EPS,
                            scalar2=None, op0=OP.add)
    ybits = statp.tile([1, 512], I32, tag="ybits", name="ybits")
    nc.vector.tensor_scalar(out=ybits[:, :], in0=ve[:, :].bitcast(I32),
                            scalar1=1, scalar2=None,
                            op0=OP.arith_shift_right)
    nc.vector.tensor_scalar(out=ybits[:, :], in0=ybits[:, :],
                            scalar1=-1, scalar2=None, op0=OP.bitwise_xor)
    y = statp.tile([1, 512], F32, tag="y", name="y")
    nc.vector.tensor_scalar(out=y[:, :].bitcast(I32), in0=ybits[:, :],
                            scalar1=QUAKE_C + 1, scalar2=None, op0=OP.add)
    t = statp.tile([1, 512], F32, tag="t", name="t")
    for _ in range(1):
        nc.vector.tensor_mul(t[:, :], y[:, :], y[:, :])
        nc.vector.tensor_mul(t[:, :], t[:, :], ve[:, :])
        nc.vector.tensor_scalar(out=t[:, :], in0=t[:, :], scalar1=-0.5,
                                scalar2=1.5, op0=OP.mult, op1=OP.add)
        nc.vector.tensor_mul(y[:, :], y[:, :], t[:, :])
    A = statp.tile([1, 512], F32R, tag="A", name="A")
    nc.vector.tensor_mul(A[:, :], y[:, :], gamma_sb[:, :])
    B = statp.tile([1, 512], F32R, tag="Bt", name="Bt")
    nc.vector.tensor_mul(B[:, :], mean[:, :], A[:, :])
    nc.vector.tensor_sub(B[:, :], beta_sb[:, :], B[:, :])
    a_ps = psum.tile([128, 512], F32, tag="score_ps", name="a_ps", bufs=2)
    b_ps = psum.tile([128, 512], F32, tag="u_ps", name="b_ps")
    nc.tensor.matmul(a_ps[:, :], ones_row_r, A[:, :],
                     start=True, stop=True)
    nc.tensor.matmul(b_ps[:, :], ones_row_r, B[:, :],
                     start=True, stop=True)
    t2 = tmpp.tile([128, SC, 512], F32, tag="t2", name="t2", bufs=1)
    outr = out[:, :].rearrange("(r p) c -> p r c", p=128)
    hh = SC // 2
    for h0, h1 in ((0, hh), (hh, SC)):
        in0, in1 = bass.broadcast_tensor_aps(outA[:, h0:h1, :],
                                             a_ps[:, :].unsqueeze(1))
        nc.vector.tensor_mul(t2[:, h0:h1, :], in0, in1)
        in0, in1 = bass.broadcast_tensor_aps(t2[:, h0:h1, :],
                                             b_ps[:, :].unsqueeze(1))
        nc.vector.tensor_add(t2[:, h0:h1, :], in0, in1)
        nc.sync.dma_start(outr[:, h0:h1, :], t2[:, h0:h1, :])
    ctx.close()


def build_nc(reps=1, parts=("prep", "attn"), with_tail=None):
    nc = bacc.Bacc("TRN2", target_bir_lowering=False, debug=False)
    x = nc.dram_tensor("x", [S, H], F32, kind="ExternalInput")
    wq = nc.dram_tensor("wq", [H, H], F32, kind="ExternalInput")
    bq = nc.dram_tensor("bq", [H], F32, kind="ExternalInput")
    wk = nc.dram_tensor("wk", [H, H], F32, kind="ExternalInput")
    bk = nc.dram_tensor("bk", [H], F32, kind="ExternalInput")
    wv = nc.dram_tensor("wv", [H, H], F32, kind="ExternalInput")
    bv = nc.dram_tensor("bv", [H], F32, kind="ExternalInput")
    gamma = nc.dram_tensor("gamma", [H], F32, kind="ExternalInput")
    beta = nc.dram_tensor("beta", [H], F32, kind="ExternalInput")
    out = nc.dram_tensor("out", [S, H], F32, kind="ExternalOutput")
    tens = (x, wq, bq, wk, bk, wv, bv, gamma, beta, out)

    with ExitStack() as ctx:
        tc = ctx.enter_context(tile.TileContext(nc))
        if with_tail is None:
            with_tail = (reps == 1)
        if reps == 1:
            emit_body(nc, tc, ctx, tens, with_tail=with_tail, parts=parts)
        else:
            hints = (mybir.EngineType.PE, mybir.EngineType.DVE,
                     mybir.EngineType.Activation, mybir.EngineType.SP)
            with tc.For_i(0, reps, 1, hint_engines=hints):
                emit_body(nc, tc, ctx, tens, with_tail=False, parts=parts)
    nc.compile()
    return nc


_CACHED_NC = None


def kernel(**inputs):
    global _CACHED_NC
    x_full = np.ascontiguousarray(np.asarray(inputs["inputs"], dtype=np.float32))
    args = {k: np.ascontiguousarray(np.asarray(inputs[k], dtype=np.float32))
            for k in ("wq", "bq", "wk", "bk", "wv", "bv", "gamma", "beta")}
    if _CACHED_NC is None:
        _CACHED_NC = build_nc(reps=1)
    nc = _CACHED_NC
    in_maps = []
    for b in range(N_CORES):
        m = {"x": x_full[b]}
        m.update(args)
        in_maps.append(m)
    res = run_bass_kernel_spmd(nc, in_maps, list(range(N_CORES)))
    out = np.stack([res.results[b]["out"] for b in range(N_CORES)], axis=0)
    return out.astype(np.float32)

